# revision 14
# baseline (speedup 1.0000x reference)
"""Betti-matching loss kernel for Trainium2 (8 NeuronCores, SPMD).

Strategy
--------
The reference computes, per sample, 0-dim superlevel persistence diagrams of
pred=softmax(logits)[1] and of the binary target, then a rank-matching loss.

Device (one image per core; 4 pred + 4 target images = 8 cores):
  * v = sigmoid(chB - chA)      (for target cores the host feeds chA=0,
                                 chB=80*t-40 so sigmoid gives {~0, 1})
  * steepest-ascent direction field over (value, -index) lexicographic order
  * basin labels resolved by iterated directional prefix scans
    (tensor_tensor_scan along rows; transposed for columns)
  * outputs: v field + basin label field (peak pixel index per pixel)

Host:
  * finish label convergence (pointer jumping; device normally converges)
  * contract each basin to its peak; boundary-pair edges w=min(v_p,v_q)
  * Kruskal union-find over ~1k peaks -> persistence bars (exactly equal to
    the reference's pixel-level union-find diagram; validated)
  * closed-form rank matching loss, mean over batch.
"""

import numpy as np

H = W = 64
N = H * W
NROUNDS = 12
NEG = -1e30

_NC_CACHE = {}
TRACE = False          # test harness can flip this to profile
LAST_RESULTS = None    # BassKernelResults of the most recent device run


def _build_nc():
    import concourse.bass as bass
    import concourse.bacc as bacc
    import concourse.mybir as mybir
    from concourse.tile import TileContext

    f32 = mybir.dt.float32
    Alu = mybir.AluOpType
    Act = mybir.ActivationFunctionType

    nc = bacc.Bacc(None)
    x = nc.dram_tensor("x", [2, H, W], f32, kind="ExternalInput")
    v_out = nc.dram_tensor("v_out", [H, W], f32, kind="ExternalOutput")
    li_out = nc.dram_tensor("li_out", [H, W], f32, kind="ExternalOutput")

    with TileContext(nc) as tc:
        with tc.tile_pool(name="main", bufs=1) as pool:
            T = lambda name: pool.tile([H, W], f32, tag=name, name=name)

            xw = pool.tile([H, 2 * W], f32, tag="xw", name="xw")
            v = T("v")
            nc.gpsimd.dma_start(
                xw[:], bass.AP(x, 0, [[W, H], [H * W, 2], [1, W]])
            )
            d = T("d")
            nc.vector.tensor_tensor(d[:], xw[:, W : 2 * W], xw[:, 0:W], Alu.subtract)
            nc.scalar.activation(v[:], d[:], Act.Sigmoid)

            # neighbor-shifted value fields, NEG at borders
            vN = T("vN")
            vS = T("vS")
            vW = T("vW")
            vE = T("vE")
            nc.vector.memset(vN[:], NEG)
            nc.gpsimd.dma_start(vN[1:H, :], v[0 : H - 1, :])
            nc.vector.memset(vS[:], NEG)
            nc.gpsimd.dma_start(vS[0 : H - 1, :], v[1:H, :])
            nc.vector.memset(vW[:, 0:1], NEG)
            nc.vector.tensor_copy(vW[:, 1:W], v[:, 0 : W - 1])
            nc.vector.memset(vE[:, W - 1 : W], NEG)
            nc.vector.tensor_copy(vE[:, 0 : W - 1], v[:, 1:W])

            # lexicographic argmax over (value, -index): candidates in
            # increasing index order N, W, self, E, S with strict >
            bV = T("bV")
            bD = T("bD")
            nc.vector.tensor_copy(bV[:], vN[:])
            nc.vector.memset(bD[:], 1.0)
            consts = {}
            for code in (0.0, 2.0, 3.0, 4.0):
                c = T(f"k{int(code)}")
                nc.vector.memset(c[:], code)
                consts[code] = c
            t = pool.tile([H, W], mybir.dt.uint32, tag="t", name="t")
            for cand, code in ((vW, 2.0), (v, 0.0), (vE, 3.0), (vS, 4.0)):
                nc.vector.tensor_tensor(t[:], cand[:], bV[:], Alu.is_gt)
                nc.vector.copy_predicated(bV[:], t[:], cand[:])
                nc.vector.copy_predicated(bD[:], t[:], consts[code][:])

            # masks per direction + complements
            masks = {}
            for code, name in ((1.0, "mN"), (2.0, "mW"), (3.0, "mE"), (4.0, "mS")):
                m = T(name)
                nc.vector.tensor_scalar(m[:], bD[:], code, None, Alu.is_equal)
                nm = T("n" + name)
                nc.vector.tensor_scalar(nm[:], m[:], -1.0, 1.0, Alu.mult, Alu.add)
                masks[name] = m
                masks["n" + name] = nm

            def transpose64(dst, src):
                for pi in (0, 1):
                    for fi in (0, 1):
                        nc.vector.transpose(
                            dst[32 * fi : 32 * fi + 32, 32 * pi : 32 * pi + 32],
                            src[32 * pi : 32 * pi + 32, 32 * fi : 32 * fi + 32],
                        )

            # transposed masks for the column-space scans
            mNT = T("mNT")
            nmNT = T("nmNT")
            mST = T("mST")
            nmST = T("nmST")
            transpose64(mNT, masks["mN"])
            transpose64(mST, masks["mS"])
            nc.vector.tensor_scalar(nmNT[:], mNT[:], -1.0, 1.0, Alu.mult, Alu.add)
            nc.vector.tensor_scalar(nmST[:], mST[:], -1.0, 1.0, Alu.mult, Alu.add)

            # label init = own pixel index
            ii = pool.tile([H, W], mybir.dt.int32, tag="ii", name="ii")
            nc.gpsimd.iota(ii[:], pattern=[[1, W]], base=0, channel_multiplier=W)
            Li = T("Li")
            nc.vector.tensor_copy(Li[:], ii[:])

            tmp = T("tmp")
            Lb = T("Lb")
            LiT = T("LiT")
            LbT = T("LbT")
            mW_, nmW_ = masks["mW"], masks["nmW"]
            mE_, nmE_ = masks["mE"], masks["nmE"]

            for _ in range(NROUNDS):
                # W-chains: left->right scan along rows
                nc.vector.tensor_tensor(tmp[:], Li[:], nmW_[:], Alu.mult)
                nc.vector.tensor_tensor_scan(
                    Lb[:], mW_[:], tmp[:], 0.0, Alu.mult, Alu.add
                )
                # E-chains: right->left scan (reversed views)
                nc.vector.tensor_tensor(tmp[:], Lb[:], nmE_[:], Alu.mult)
                nc.vector.tensor_tensor_scan(
                    Li[:, ::-1], mE_[:, ::-1], tmp[:, ::-1], 0.0, Alu.mult, Alu.add
                )
                # column space
                transpose64(LiT, Li)
                # N-chains: in transposed space, left->right
                nc.vector.tensor_tensor(tmp[:], LiT[:], nmNT[:], Alu.mult)
                nc.vector.tensor_tensor_scan(
                    LbT[:], mNT[:], tmp[:], 0.0, Alu.mult, Alu.add
                )
                # S-chains: right->left in transposed space
                nc.vector.tensor_tensor(tmp[:], LbT[:], nmST[:], Alu.mult)
                nc.vector.tensor_tensor_scan(
                    LiT[:, ::-1], mST[:, ::-1], tmp[:, ::-1], 0.0, Alu.mult, Alu.add
                )
                transpose64(Li, LiT)

            nc.gpsimd.dma_start(v_out[:], v[:])
            nc.gpsimd.dma_start(li_out[:], Li[:])

    return nc


def _run_device(xs):
    """xs: list of 8 arrays [2,H,W] f32. Returns list of (v, li) pairs."""
    from concourse.bass_utils import run_bass_kernel_spmd

    if "nc" not in _NC_CACHE:
        nc = _build_nc()
        if not nc.is_finalized():
            nc.finalize()
        _NC_CACHE["nc"] = nc
    nc = _NC_CACHE["nc"]
    res = run_bass_kernel_spmd(
        nc,
        [{"x": np.ascontiguousarray(x, dtype=np.float32)} for x in xs],
        core_ids=list(range(8)),
        trace=TRACE,
    )
    global LAST_RESULTS
    LAST_RESULTS = res
    return [(r["v_out"], r["li_out"]) for r in res.results]


# ---------------------------------------------------------------------------
# host post-processing
# ---------------------------------------------------------------------------

def _ascent_ptr(v):
    """Pointer to steepest-ascent target under (value, -index) lex order.
    Must mirror the device compare cascade bit-exactly (pure f32 compares)."""
    neg = np.float32(NEG)
    vN = np.full((H, W), neg, np.float32); vN[1:, :] = v[:-1, :]
    vS = np.full((H, W), neg, np.float32); vS[:-1, :] = v[1:, :]
    vW = np.full((H, W), neg, np.float32); vW[:, 1:] = v[:, :-1]
    vE = np.full((H, W), neg, np.float32); vE[:, :-1] = v[:, 1:]
    bV = vN.copy()
    bD = np.full((H, W), 1, np.int32)
    for cand, code in ((vW, 2), (v, 0), (vE, 3), (vS, 4)):
        take = cand > bV
        bV = np.where(take, cand, bV)
        bD = np.where(take, code, bD)
    idx = np.arange(N).reshape(H, W)
    off = np.array([0, -W, -1, 1, W])
    return (idx + off[bD]).reshape(-1)


def _resolve_labels(li, ptr):
    """Finish pointer-jumping from the device's (normally converged) labels."""
    L = li
    for _ in range(14):
        L2 = L[L]
        if np.array_equal(L2, L):
            return L
        L = L2
    # pathological fallback: resolve from raw pointers
    L = ptr
    while True:
        L2 = L[L]
        if np.array_equal(L2, L):
            return L
        L = L2


def _diagram(v, L):
    """Positive-persistence bars via basin contraction + Kruskal."""
    vf = v.reshape(-1).astype(np.float64)
    Lg = L.reshape(H, W)
    vg = v.reshape(H, W).astype(np.float64)

    eu = np.concatenate([Lg[:, :-1].reshape(-1), Lg[:-1, :].reshape(-1)])
    ev = np.concatenate([Lg[:, 1:].reshape(-1), Lg[1:, :].reshape(-1)])
    ew = np.concatenate([
        np.minimum(vg[:, :-1], vg[:, 1:]).reshape(-1),
        np.minimum(vg[:-1, :], vg[1:, :]).reshape(-1),
    ])
    m = eu != ev
    eu, ev, ew = eu[m], ev[m], ew[m]
    # one edge per unordered basin pair: keep the max weight
    lo = np.minimum(eu, ev)
    hi = np.maximum(eu, ev)
    order = np.lexsort((-ew, hi, lo))
    lo, hi, ew = lo[order], hi[order], ew[order]
    first = np.ones(len(lo), dtype=bool)
    first[1:] = (lo[1:] != lo[:-1]) | (hi[1:] != hi[:-1])
    lo, hi, ew = lo[first], hi[first], ew[first]
    # Kruskal by decreasing weight
    order = np.argsort(-ew, kind="stable")
    lo, hi, ew = lo[order], hi[order], ew[order]

    peaks = np.unique(L)
    pid = np.full(N, -1, np.int64)
    pid[peaks] = np.arange(len(peaks))
    parent = np.arange(len(peaks))
    birth = vf[peaks]

    plist = parent
    bars_b = []
    bars_d = []

    def find(i):
        while plist[i] != i:
            plist[i] = plist[plist[i]]
            i = plist[i]
        return i

    merges = 0
    need = len(peaks) - 1
    for k in range(len(ew)):
        ri = find(pid[lo[k]])
        rj = find(pid[hi[k]])
        if ri == rj:
            continue
        if birth[ri] >= birth[rj]:
            elder, young = ri, rj
        else:
            elder, young = rj, ri
        if birth[young] > ew[k]:
            bars_b.append(birth[young])
            bars_d.append(ew[k])
        plist[young] = elder
        merges += 1
        if merges == need:
            break
    vmax = vf.max()
    vmin = vf.min()
    if vmax > vmin:
        bars_b.append(vmax)
        bars_d.append(vmin)
    return np.array(bars_b), np.array(bars_d)


def _match_loss(b1, d1, b2, d2):
    p1 = b1 - d1
    p2 = b2 - d2
    o1 = np.argsort(-p1, kind="stable")
    o2 = np.argsort(-p2, kind="stable")
    b1, d1 = b1[o1], d1[o1]
    b2, d2 = b2[o2], d2[o2]
    K1, K2 = len(b1), len(b2)
    Km = min(K1, K2)
    loss = 0.0
    if Km:
        loss += np.sum((b1[:Km] - b2[:Km]) ** 2 + (d1[:Km] - d2[:Km]) ** 2)
    if K1 > Km:
        loss += 0.5 * np.sum((b1[Km:] - d1[Km:]) ** 2)
    if K2 > Km:
        loss += 0.5 * np.sum((b2[Km:] - d2[Km:]) ** 2)
    return loss


def _postprocess(v, li):
    v = np.asarray(v, np.float32).reshape(H, W)
    li = np.asarray(li).reshape(-1).astype(np.int64)
    ptr = _ascent_ptr(v)
    L = _resolve_labels(li, ptr)
    return _diagram(v, L)


def kernel(input, target):
    input = np.asarray(input, np.float32)
    target = np.asarray(target, np.float32)
    B = input.shape[0]
    assert B == 4 and input.shape == (4, 2, H, W) and target.shape == (4, H, W)

    xs = []
    for s in range(B):
        xs.append(input[s])
    for s in range(B):
        t = np.zeros((2, H, W), np.float32)
        t[1] = target[s] * np.float32(80.0) - np.float32(40.0)
        xs.append(t)

    outs = _run_device(xs)

    losses = []
    for s in range(B):
        bp, dp = _postprocess(*outs[s])
        bt, dt = _postprocess(*outs[4 + s])
        losses.append(_match_loss(bp, dp, bt, dt))
    return np.float32(np.mean(losses))


# revision 18
# speedup vs baseline: 1.3176x; 1.3176x over previous
"""Betti-matching loss kernel for Trainium2 (8 NeuronCores, SPMD).

Strategy
--------
The reference computes, per sample, 0-dim superlevel persistence diagrams of
pred=softmax(logits)[1] and of the binary target, then a rank-matching loss.

Device (one image per core; 4 pred + 4 target images = 8 cores):
  * v = sigmoid(chB - chA)      (for target cores the host feeds chA=0,
                                 chB=80*t-40 so sigmoid gives {~0, 1})
  * steepest-ascent direction field over (value, -index) lexicographic order
  * basin labels resolved by iterated directional prefix scans
    (tensor_tensor_scan along rows; transposed for columns)
  * outputs: v field + basin label field (peak pixel index per pixel)

Host:
  * finish label convergence (pointer jumping; device normally converges)
  * contract each basin to its peak; boundary-pair edges w=min(v_p,v_q)
  * Kruskal union-find over ~1k peaks -> persistence bars (exactly equal to
    the reference's pixel-level union-find diagram; validated)
  * closed-form rank matching loss, mean over batch.
"""

import numpy as np

H = W = 64
N = H * W
NROUNDS = 11
NEG = -1e30

_NC_CACHE = {}
TRACE = False          # test harness can flip this to profile
LAST_RESULTS = None    # BassKernelResults of the most recent device run


def _build_nc():
    import concourse.bass as bass
    import concourse.bacc as bacc
    import concourse.mybir as mybir
    from concourse.tile import TileContext

    f32 = mybir.dt.float32
    Alu = mybir.AluOpType
    Act = mybir.ActivationFunctionType

    from concourse import masks as masks_mod

    nc = bacc.Bacc(None)
    x = nc.dram_tensor("x", [2, H, W], f32, kind="ExternalInput")
    # packed output: cols 0:64 = v field, cols 64:128 = transposed labels
    out = nc.dram_tensor("out", [H, 2 * W], f32, kind="ExternalOutput")

    with TileContext(nc) as tc:
        with (
            tc.tile_pool(name="main", bufs=1) as pool,
            tc.tile_pool(name="psum", bufs=2, space="PSUM") as psum,
        ):
            T = lambda name: pool.tile([H, W], f32, tag=name, name=name)

            ident = T("ident")
            masks_mod.make_identity(nc, ident[:])

            pack = pool.tile([H, 2 * W], f32, tag="pack", name="pack")
            v = pack[:, 0:W]

            xw = pool.tile([H, 2 * W], f32, tag="xw", name="xw")
            nc.gpsimd.dma_start(
                xw[:], bass.AP(x, 0, [[W, H], [H * W, 2], [1, W]])
            )
            d = T("d")
            nc.vector.tensor_tensor(d[:], xw[:, W : 2 * W], xw[:, 0:W], Alu.subtract)
            nc.scalar.activation(v, d[:], Act.Sigmoid)

            # neighbor-shifted value fields, NEG at borders
            vN = T("vN")
            vS = T("vS")
            vW = T("vW")
            vE = T("vE")
            nc.gpsimd.memset(vN[:], NEG)
            nc.gpsimd.dma_start(vN[1:H, :], v[0 : H - 1, :])
            nc.gpsimd.memset(vS[:], NEG)
            nc.gpsimd.dma_start(vS[0 : H - 1, :], v[1:H, :])
            nc.vector.memset(vW[:, 0:1], NEG)
            nc.vector.tensor_copy(vW[:, 1:W], v[:, 0 : W - 1])
            nc.vector.memset(vE[:, W - 1 : W], NEG)
            nc.vector.tensor_copy(vE[:, 0 : W - 1], v[:, 1:W])

            # lexicographic argmax over (value, -index): candidates in
            # increasing index order N, W, self, E, S with strict >
            bV = T("bV")
            bD = T("bD")
            nc.vector.tensor_copy(bV[:], vN[:])
            nc.gpsimd.memset(bD[:], 1.0)
            consts = {}
            for code in (0.0, 2.0, 3.0, 4.0):
                c = T(f"k{int(code)}")
                nc.gpsimd.memset(c[:], code)
                consts[code] = c
            t = pool.tile([H, W], mybir.dt.uint32, tag="t", name="t")
            for cand, code in ((vW, 2.0), (v, 0.0), (vE, 3.0), (vS, 4.0)):
                nc.vector.tensor_tensor(t[:], cand[:], bV[:], Alu.is_gt)
                nc.vector.copy_predicated(bV[:], t[:], cand[:])
                nc.vector.copy_predicated(bD[:], t[:], consts[code][:])

            # row-space masks + complements
            dirmask = {}
            for code, name in ((2.0, "mW"), (3.0, "mE")):
                m = T(name)
                nc.vector.tensor_scalar(m[:], bD[:], code, None, Alu.is_equal)
                nm = T("n" + name)
                nc.vector.tensor_scalar(nm[:], m[:], -1.0, 1.0, Alu.mult, Alu.add)
                dirmask[name] = m
                dirmask["n" + name] = nm
            # col-space masks from PE-transposed direction field
            bDT = psum.tile([H, W], f32, tag="bDT", name="bDT")
            nc.tensor.transpose(bDT[:], bD[:], ident[:])
            for code, name in ((1.0, "mNT"), (4.0, "mST")):
                m = T(name)
                nc.vector.tensor_scalar(m[:], bDT[:], code, None, Alu.is_equal)
                nm = T("n" + name)
                nc.vector.tensor_scalar(nm[:], m[:], -1.0, 1.0, Alu.mult, Alu.add)
                dirmask[name] = m
                dirmask["n" + name] = nm
            mW_, nmW_ = dirmask["mW"], dirmask["nmW"]
            mE_, nmE_ = dirmask["mE"], dirmask["nmE"]
            mNT_, nmNT_ = dirmask["mNT"], dirmask["nmNT"]
            mST_, nmST_ = dirmask["mST"], dirmask["nmST"]

            # label init = own pixel index
            ii = pool.tile([H, W], mybir.dt.int32, tag="ii", name="ii")
            nc.gpsimd.iota(ii[:], pattern=[[1, W]], base=0, channel_multiplier=W)
            Li0 = T("Li0")
            nc.vector.tensor_copy(Li0[:], ii[:])

            tmp = T("tmp")
            La = T("La")
            Lb = T("Lb")
            Lc = T("Lc")
            Ld = pack[:, W : 2 * W]
            cur = Li0[:]
            for r in range(NROUNDS):
                # W-chains: left->right scan along rows
                nc.vector.tensor_tensor(tmp[:], cur, nmW_[:], Alu.mult)
                nc.vector.tensor_tensor_scan(
                    La[:], mW_[:], tmp[:], 0.0, Alu.mult, Alu.add
                )
                # E-chains: right->left scan (reversed views)
                nc.vector.tensor_tensor(tmp[:], La[:], nmE_[:], Alu.mult)
                nc.vector.tensor_tensor_scan(
                    Lb[:, ::-1], mE_[:, ::-1], tmp[:, ::-1], 0.0, Alu.mult, Alu.add
                )
                # to column space on the PE
                psT = psum.tile([H, W], f32, tag="psT", name="psT")
                nc.tensor.transpose(psT[:], Lb[:], ident[:])
                # N-chains: left->right in transposed space
                nc.vector.tensor_tensor(tmp[:], psT[:], nmNT_[:], Alu.mult)
                nc.vector.tensor_tensor_scan(
                    Lc[:], mNT_[:], tmp[:], 0.0, Alu.mult, Alu.add
                )
                # S-chains: right->left in transposed space
                nc.vector.tensor_tensor(tmp[:], Lc[:], nmST_[:], Alu.mult)
                last = r == NROUNDS - 1
                sout = Ld if last else T("Ls")
                nc.vector.tensor_tensor_scan(
                    sout[:, ::-1], mST_[:, ::-1], tmp[:, ::-1],
                    0.0, Alu.mult, Alu.add,
                )
                if not last:
                    # back to row space for the next round
                    psR = psum.tile([H, W], f32, tag="psR", name="psR")
                    nc.tensor.transpose(psR[:], sout[:], ident[:])
                    cur = psR[:]

            nc.gpsimd.dma_start(out[:], pack[:])

    return nc


def _run_device(xs):
    """xs: list of 8 arrays [2,H,W] f32. Returns list of (v, li) pairs."""
    from concourse.bass_utils import run_bass_kernel_spmd

    if "nc" not in _NC_CACHE:
        nc = _build_nc()
        if not nc.is_finalized():
            nc.finalize()
        _NC_CACHE["nc"] = nc
    nc = _NC_CACHE["nc"]
    res = run_bass_kernel_spmd(
        nc,
        [{"x": np.ascontiguousarray(x, dtype=np.float32)} for x in xs],
        core_ids=list(range(8)),
        trace=TRACE,
    )
    global LAST_RESULTS
    LAST_RESULTS = res
    # packed output: cols 0:64 = v, cols 64:128 = labels in transposed layout
    return [
        (r["out"][:, 0:W], np.ascontiguousarray(r["out"][:, W : 2 * W].T))
        for r in res.results
    ]


# ---------------------------------------------------------------------------
# host post-processing
# ---------------------------------------------------------------------------

def _ascent_ptr(v):
    """Pointer to steepest-ascent target under (value, -index) lex order.
    Must mirror the device compare cascade bit-exactly (pure f32 compares)."""
    neg = np.float32(NEG)
    vN = np.full((H, W), neg, np.float32); vN[1:, :] = v[:-1, :]
    vS = np.full((H, W), neg, np.float32); vS[:-1, :] = v[1:, :]
    vW = np.full((H, W), neg, np.float32); vW[:, 1:] = v[:, :-1]
    vE = np.full((H, W), neg, np.float32); vE[:, :-1] = v[:, 1:]
    bV = vN.copy()
    bD = np.full((H, W), 1, np.int32)
    for cand, code in ((vW, 2), (v, 0), (vE, 3), (vS, 4)):
        take = cand > bV
        bV = np.where(take, cand, bV)
        bD = np.where(take, code, bD)
    idx = np.arange(N).reshape(H, W)
    off = np.array([0, -W, -1, 1, W])
    return (idx + off[bD]).reshape(-1)


def _resolve_labels(li, ptr):
    """Finish pointer-jumping from the device's (normally converged) labels,
    then verify against the ascent forest; fall back to exact pointer
    resolution if the device field is inconsistent."""
    L = li
    for _ in range(14):
        L2 = L[L]
        if np.array_equal(L2, L):
            break
        L = L2
    # validity: constant along ascent edges, and ascent roots self-labeled
    ok = np.array_equal(L, L[ptr])
    if ok:
        roots = ptr == np.arange(N)
        ok = np.array_equal(L[roots], np.arange(N)[roots])
    if ok:
        return L
    L = ptr
    while True:
        L2 = L[L]
        if np.array_equal(L2, L):
            return L
        L = L2


def _diagram(v, L):
    """Positive-persistence bars via basin contraction + Kruskal."""
    vf = v.reshape(-1).astype(np.float64)
    Lg = L.reshape(H, W)
    vg = v.reshape(H, W).astype(np.float64)

    eu = np.concatenate([Lg[:, :-1].reshape(-1), Lg[:-1, :].reshape(-1)])
    ev = np.concatenate([Lg[:, 1:].reshape(-1), Lg[1:, :].reshape(-1)])
    ew = np.concatenate([
        np.minimum(vg[:, :-1], vg[:, 1:]).reshape(-1),
        np.minimum(vg[:-1, :], vg[1:, :]).reshape(-1),
    ])
    m = eu != ev
    eu, ev, ew = eu[m], ev[m], ew[m]
    # one edge per unordered basin pair: keep the max weight
    lo = np.minimum(eu, ev)
    hi = np.maximum(eu, ev)
    order = np.lexsort((-ew, hi, lo))
    lo, hi, ew = lo[order], hi[order], ew[order]
    first = np.ones(len(lo), dtype=bool)
    first[1:] = (lo[1:] != lo[:-1]) | (hi[1:] != hi[:-1])
    lo, hi, ew = lo[first], hi[first], ew[first]
    # Kruskal by decreasing weight
    order = np.argsort(-ew, kind="stable")
    lo, hi, ew = lo[order], hi[order], ew[order]

    peaks = np.unique(L)
    pid = np.full(N, -1, np.int64)
    pid[peaks] = np.arange(len(peaks))
    parent = np.arange(len(peaks))
    birth = vf[peaks]

    plist = parent
    bars_b = []
    bars_d = []

    def find(i):
        while plist[i] != i:
            plist[i] = plist[plist[i]]
            i = plist[i]
        return i

    merges = 0
    need = len(peaks) - 1
    for k in range(len(ew)):
        ri = find(pid[lo[k]])
        rj = find(pid[hi[k]])
        if ri == rj:
            continue
        if birth[ri] >= birth[rj]:
            elder, young = ri, rj
        else:
            elder, young = rj, ri
        if birth[young] > ew[k]:
            bars_b.append(birth[young])
            bars_d.append(ew[k])
        plist[young] = elder
        merges += 1
        if merges == need:
            break
    vmax = vf.max()
    vmin = vf.min()
    if vmax > vmin:
        bars_b.append(vmax)
        bars_d.append(vmin)
    return np.array(bars_b), np.array(bars_d)


def _match_loss(b1, d1, b2, d2):
    p1 = b1 - d1
    p2 = b2 - d2
    o1 = np.argsort(-p1, kind="stable")
    o2 = np.argsort(-p2, kind="stable")
    b1, d1 = b1[o1], d1[o1]
    b2, d2 = b2[o2], d2[o2]
    K1, K2 = len(b1), len(b2)
    Km = min(K1, K2)
    loss = 0.0
    if Km:
        loss += np.sum((b1[:Km] - b2[:Km]) ** 2 + (d1[:Km] - d2[:Km]) ** 2)
    if K1 > Km:
        loss += 0.5 * np.sum((b1[Km:] - d1[Km:]) ** 2)
    if K2 > Km:
        loss += 0.5 * np.sum((b2[Km:] - d2[Km:]) ** 2)
    return loss


def _postprocess(v, li):
    v = np.asarray(v, np.float32).reshape(H, W)
    li = np.asarray(li).reshape(-1).astype(np.int64)
    ptr = _ascent_ptr(v)
    L = _resolve_labels(li, ptr)
    return _diagram(v, L)


def kernel(input, target):
    input = np.asarray(input, np.float32)
    target = np.asarray(target, np.float32)
    B = input.shape[0]
    assert B == 4 and input.shape == (4, 2, H, W) and target.shape == (4, H, W)

    xs = []
    for s in range(B):
        xs.append(input[s])
    for s in range(B):
        t = np.zeros((2, H, W), np.float32)
        t[1] = target[s] * np.float32(80.0) - np.float32(40.0)
        xs.append(t)

    outs = _run_device(xs)

    losses = []
    for s in range(B):
        bp, dp = _postprocess(*outs[s])
        bt, dt = _postprocess(*outs[4 + s])
        losses.append(_match_loss(bp, dp, bt, dt))
    return np.float32(np.mean(losses))


# revision 23
# speedup vs baseline: 1.5598x; 1.1838x over previous
"""Betti-matching loss kernel for Trainium2 (8 NeuronCores, SPMD).

Strategy
--------
The reference computes, per sample, 0-dim superlevel persistence diagrams of
pred=softmax(logits)[1] and of the binary target, then a rank-matching loss.

Device (one image per core; 4 pred + 4 target images = 8 cores):
  * v = sigmoid(chB - chA)      (for target cores the host feeds chA=0,
                                 chB=80*t-40 so sigmoid gives {~0, 1})
  * steepest-ascent direction field over (value, -index) lexicographic order
  * basin labels resolved by iterated directional prefix scans
    (tensor_tensor_scan along rows; transposed for columns)
  * outputs: v field + basin label field (peak pixel index per pixel)

Host:
  * finish label convergence (pointer jumping; device normally converges)
  * contract each basin to its peak; boundary-pair edges w=min(v_p,v_q)
  * Kruskal union-find over ~1k peaks -> persistence bars (exactly equal to
    the reference's pixel-level union-find diagram; validated)
  * closed-form rank matching loss, mean over batch.
"""

import numpy as np

H = W = 64
N = H * W
NROUNDS = 9
NEG = -1e30
FALLBACKS = 0  # images where the host had to re-resolve labels from scratch

_NC_CACHE = {}
TRACE = False          # test harness can flip this to profile
LAST_RESULTS = None    # BassKernelResults of the most recent device run


def _build_nc():
    import concourse.bass as bass
    import concourse.bacc as bacc
    import concourse.mybir as mybir
    from concourse.tile import TileContext

    f32 = mybir.dt.float32
    Alu = mybir.AluOpType
    Act = mybir.ActivationFunctionType

    from concourse import masks as masks_mod

    nc = bacc.Bacc(None)
    x = nc.dram_tensor("x", [2, H, W], f32, kind="ExternalInput")
    # packed output: cols 0:64 = v field, cols 64:128 = transposed labels
    out = nc.dram_tensor("out", [H, 2 * W], f32, kind="ExternalOutput")

    with TileContext(nc) as tc:
        with (
            tc.tile_pool(name="main", bufs=1) as pool,
            tc.tile_pool(name="psum", bufs=2, space="PSUM") as psum,
        ):
            T = lambda name: pool.tile([H, W], f32, tag=name, name=name)

            # gpsimd work first so its queue drains during the input DMA:
            # iota (label init), border fills, cascade constants, identity
            ii = pool.tile([H, W], mybir.dt.int32, tag="ii", name="ii")
            nc.gpsimd.iota(ii[:], pattern=[[1, W]], base=0, channel_multiplier=W)
            vN = T("vN")
            vS = T("vS")
            nc.gpsimd.memset(vN[:], NEG)
            nc.gpsimd.memset(vS[:], NEG)
            bD = T("bD")
            nc.gpsimd.memset(bD[:], 1.0)
            consts = {}
            for code in (0.0, 2.0, 3.0, 4.0):
                c = T(f"k{int(code)}")
                nc.gpsimd.memset(c[:], code)
                consts[code] = c
            ident = T("ident")
            masks_mod.make_identity(nc, ident[:])

            pack = pool.tile([H, 2 * W], f32, tag="pack", name="pack")
            v = pack[:, 0:W]

            xw = pool.tile([H, 2 * W], f32, tag="xw", name="xw")
            nc.sync.dma_start(
                xw[:], bass.AP(x, 0, [[W, H], [H * W, 2], [1, W]])
            )
            # warm the sigmoid table on ACT while the input DMA is in flight
            warm = pool.tile([H, 1], f32, tag="warm", name="warm")
            nc.vector.memset(warm[:], 0.0)
            nc.scalar.activation(warm[:], warm[:], Act.Sigmoid)

            d = T("d")
            nc.vector.tensor_tensor(d[:], xw[:, W : 2 * W], xw[:, 0:W], Alu.subtract)
            nc.scalar.activation(v, d[:], Act.Sigmoid)

            # neighbor-shifted value fields, NEG at borders
            vW = T("vW")
            vE = T("vE")
            nc.sync.dma_start(vN[1:H, :], v[0 : H - 1, :])
            nc.sync.dma_start(vS[0 : H - 1, :], v[1:H, :])
            nc.vector.memset(vW[:, 0:1], NEG)
            nc.vector.tensor_copy(vW[:, 1:W], v[:, 0 : W - 1])
            nc.vector.memset(vE[:, W - 1 : W], NEG)
            nc.vector.tensor_copy(vE[:, 0 : W - 1], v[:, 1:W])

            # lexicographic argmax over (value, -index): candidates in
            # increasing index order N, W, self, E, S with strict >
            bV = T("bV")
            nc.vector.tensor_copy(bV[:], vN[:])
            t = pool.tile([H, W], mybir.dt.uint32, tag="t", name="t")
            for cand, code in ((vW, 2.0), (v, 0.0), (vE, 3.0), (vS, 4.0)):
                nc.vector.tensor_tensor(t[:], cand[:], bV[:], Alu.is_gt)
                nc.vector.copy_predicated(bV[:], t[:], cand[:])
                nc.vector.copy_predicated(bD[:], t[:], consts[code][:])

            # row-space masks + complements
            dirmask = {}
            for code, name in ((2.0, "mW"), (3.0, "mE")):
                m = T(name)
                nc.vector.tensor_scalar(m[:], bD[:], code, None, Alu.is_equal)
                nm = T("n" + name)
                nc.vector.tensor_scalar(nm[:], m[:], -1.0, 1.0, Alu.mult, Alu.add)
                dirmask[name] = m
                dirmask["n" + name] = nm
            # col-space masks from PE-transposed direction field
            bDT = psum.tile([H, W], f32, tag="bDT", name="bDT")
            nc.tensor.transpose(bDT[:], bD[:], ident[:])
            for code, name in ((1.0, "mNT"), (4.0, "mST")):
                m = T(name)
                nc.vector.tensor_scalar(m[:], bDT[:], code, None, Alu.is_equal)
                nm = T("n" + name)
                nc.vector.tensor_scalar(nm[:], m[:], -1.0, 1.0, Alu.mult, Alu.add)
                dirmask[name] = m
                dirmask["n" + name] = nm
            mW_, nmW_ = dirmask["mW"], dirmask["nmW"]
            mE_, nmE_ = dirmask["mE"], dirmask["nmE"]
            mNT_, nmNT_ = dirmask["mNT"], dirmask["nmNT"]
            mST_, nmST_ = dirmask["mST"], dirmask["nmST"]

            # label init = own pixel index
            Li0 = T("Li0")
            nc.vector.tensor_copy(Li0[:], ii[:])

            tmp = T("tmp")
            La = T("La")
            Lb = T("Lb")
            Lc = T("Lc")
            Ld = pack[:, W : 2 * W]
            cur = Li0[:]
            for r in range(NROUNDS):
                # W-chains: left->right scan along rows
                nc.vector.tensor_tensor(tmp[:], cur, nmW_[:], Alu.mult)
                nc.vector.tensor_tensor_scan(
                    La[:], mW_[:], tmp[:], 0.0, Alu.mult, Alu.add
                )
                # E-chains: right->left scan (reversed views)
                nc.vector.tensor_tensor(tmp[:], La[:], nmE_[:], Alu.mult)
                nc.vector.tensor_tensor_scan(
                    Lb[:, ::-1], mE_[:, ::-1], tmp[:, ::-1], 0.0, Alu.mult, Alu.add
                )
                # to column space on the PE
                psT = psum.tile([H, W], f32, tag="psT", name="psT")
                nc.tensor.transpose(psT[:], Lb[:], ident[:])
                # N-chains: left->right in transposed space
                nc.vector.tensor_tensor(tmp[:], psT[:], nmNT_[:], Alu.mult)
                nc.vector.tensor_tensor_scan(
                    Lc[:], mNT_[:], tmp[:], 0.0, Alu.mult, Alu.add
                )
                # S-chains: right->left in transposed space
                nc.vector.tensor_tensor(tmp[:], Lc[:], nmST_[:], Alu.mult)
                last = r == NROUNDS - 1
                sout = Ld if last else T("Ls")
                nc.vector.tensor_tensor_scan(
                    sout[:, ::-1], mST_[:, ::-1], tmp[:, ::-1],
                    0.0, Alu.mult, Alu.add,
                )
                if not last:
                    # back to row space for the next round
                    psR = psum.tile([H, W], f32, tag="psR", name="psR")
                    nc.tensor.transpose(psR[:], sout[:], ident[:])
                    cur = psR[:]

            nc.sync.dma_start(out[:], pack[:])

    return nc


def _run_device(xs):
    """xs: list of 8 arrays [2,H,W] f32. Returns list of (v, li) pairs."""
    from concourse.bass_utils import run_bass_kernel_spmd

    if "nc" not in _NC_CACHE:
        nc = _build_nc()
        if not nc.is_finalized():
            nc.finalize()
        _NC_CACHE["nc"] = nc
    nc = _NC_CACHE["nc"]
    res = run_bass_kernel_spmd(
        nc,
        [{"x": np.ascontiguousarray(x, dtype=np.float32)} for x in xs],
        core_ids=list(range(8)),
        trace=TRACE,
    )
    global LAST_RESULTS
    LAST_RESULTS = res
    # packed output: cols 0:64 = v, cols 64:128 = labels in transposed layout
    return [
        (r["out"][:, 0:W], np.ascontiguousarray(r["out"][:, W : 2 * W].T))
        for r in res.results
    ]


# ---------------------------------------------------------------------------
# host post-processing
# ---------------------------------------------------------------------------

def _ascent_ptr(v):
    """Pointer to steepest-ascent target under (value, -index) lex order.
    Must mirror the device compare cascade bit-exactly (pure f32 compares)."""
    neg = np.float32(NEG)
    vN = np.full((H, W), neg, np.float32); vN[1:, :] = v[:-1, :]
    vS = np.full((H, W), neg, np.float32); vS[:-1, :] = v[1:, :]
    vW = np.full((H, W), neg, np.float32); vW[:, 1:] = v[:, :-1]
    vE = np.full((H, W), neg, np.float32); vE[:, :-1] = v[:, 1:]
    bV = vN.copy()
    bD = np.full((H, W), 1, np.int32)
    for cand, code in ((vW, 2), (v, 0), (vE, 3), (vS, 4)):
        take = cand > bV
        bV = np.where(take, cand, bV)
        bD = np.where(take, code, bD)
    idx = np.arange(N).reshape(H, W)
    off = np.array([0, -W, -1, 1, W])
    return (idx + off[bD]).reshape(-1)


def _resolve_labels(li, ptr):
    """Finish pointer-jumping from the device's (normally converged) labels,
    then verify against the ascent forest; fall back to exact pointer
    resolution if the device field is inconsistent."""
    L = li
    for _ in range(14):
        L2 = L[L]
        if np.array_equal(L2, L):
            break
        L = L2
    # validity: constant along ascent edges, and ascent roots self-labeled
    ok = np.array_equal(L, L[ptr])
    if ok:
        roots = ptr == np.arange(N)
        ok = np.array_equal(L[roots], np.arange(N)[roots])
    if ok:
        return L
    global FALLBACKS
    FALLBACKS += 1
    L = ptr
    while True:
        L2 = L[L]
        if np.array_equal(L2, L):
            return L
        L = L2


def _diagram(v, L):
    """Positive-persistence bars via basin contraction + Kruskal."""
    vf = v.reshape(-1).astype(np.float64)
    Lg = L.reshape(H, W)
    vg = v.reshape(H, W).astype(np.float64)

    eu = np.concatenate([Lg[:, :-1].reshape(-1), Lg[:-1, :].reshape(-1)])
    ev = np.concatenate([Lg[:, 1:].reshape(-1), Lg[1:, :].reshape(-1)])
    ew = np.concatenate([
        np.minimum(vg[:, :-1], vg[:, 1:]).reshape(-1),
        np.minimum(vg[:-1, :], vg[1:, :]).reshape(-1),
    ])
    m = eu != ev
    eu, ev, ew = eu[m], ev[m], ew[m]
    # one edge per unordered basin pair: keep the max weight
    lo = np.minimum(eu, ev)
    hi = np.maximum(eu, ev)
    order = np.lexsort((-ew, hi, lo))
    lo, hi, ew = lo[order], hi[order], ew[order]
    first = np.ones(len(lo), dtype=bool)
    first[1:] = (lo[1:] != lo[:-1]) | (hi[1:] != hi[:-1])
    lo, hi, ew = lo[first], hi[first], ew[first]
    # Kruskal by decreasing weight
    order = np.argsort(-ew, kind="stable")
    lo, hi, ew = lo[order], hi[order], ew[order]

    peaks = np.unique(L)
    pid = np.full(N, -1, np.int64)
    pid[peaks] = np.arange(len(peaks))
    parent = np.arange(len(peaks))
    birth = vf[peaks]

    plist = parent
    bars_b = []
    bars_d = []

    def find(i):
        while plist[i] != i:
            plist[i] = plist[plist[i]]
            i = plist[i]
        return i

    merges = 0
    need = len(peaks) - 1
    for k in range(len(ew)):
        ri = find(pid[lo[k]])
        rj = find(pid[hi[k]])
        if ri == rj:
            continue
        if birth[ri] >= birth[rj]:
            elder, young = ri, rj
        else:
            elder, young = rj, ri
        if birth[young] > ew[k]:
            bars_b.append(birth[young])
            bars_d.append(ew[k])
        plist[young] = elder
        merges += 1
        if merges == need:
            break
    vmax = vf.max()
    vmin = vf.min()
    if vmax > vmin:
        bars_b.append(vmax)
        bars_d.append(vmin)
    return np.array(bars_b), np.array(bars_d)


def _match_loss(b1, d1, b2, d2):
    p1 = b1 - d1
    p2 = b2 - d2
    o1 = np.argsort(-p1, kind="stable")
    o2 = np.argsort(-p2, kind="stable")
    b1, d1 = b1[o1], d1[o1]
    b2, d2 = b2[o2], d2[o2]
    K1, K2 = len(b1), len(b2)
    Km = min(K1, K2)
    loss = 0.0
    if Km:
        loss += np.sum((b1[:Km] - b2[:Km]) ** 2 + (d1[:Km] - d2[:Km]) ** 2)
    if K1 > Km:
        loss += 0.5 * np.sum((b1[Km:] - d1[Km:]) ** 2)
    if K2 > Km:
        loss += 0.5 * np.sum((b2[Km:] - d2[Km:]) ** 2)
    return loss


def _postprocess(v, li):
    v = np.asarray(v, np.float32).reshape(H, W)
    li = np.asarray(li).reshape(-1).astype(np.int64)
    ptr = _ascent_ptr(v)
    L = _resolve_labels(li, ptr)
    return _diagram(v, L)


def kernel(input, target):
    input = np.asarray(input, np.float32)
    target = np.asarray(target, np.float32)
    B = input.shape[0]
    assert B == 4 and input.shape == (4, 2, H, W) and target.shape == (4, H, W)

    xs = []
    for s in range(B):
        xs.append(input[s])
    for s in range(B):
        t = np.zeros((2, H, W), np.float32)
        t[1] = target[s] * np.float32(80.0) - np.float32(40.0)
        xs.append(t)

    outs = _run_device(xs)

    losses = []
    for s in range(B):
        bp, dp = _postprocess(*outs[s])
        bt, dt = _postprocess(*outs[4 + s])
        losses.append(_match_loss(bp, dp, bt, dt))
    return np.float32(np.mean(losses))


# revision 26
# speedup vs baseline: 1.6860x; 1.0809x over previous
"""Betti-matching loss kernel for Trainium2 (8 NeuronCores, SPMD).

Strategy
--------
The reference computes, per sample, 0-dim superlevel persistence diagrams of
pred=softmax(logits)[1] and of the binary target, then a rank-matching loss.

Device (one image per core; 4 pred + 4 target images = 8 cores):
  * v = sigmoid(chB - chA)      (for target cores the host feeds chA=0,
                                 chB=80*t-40 so sigmoid gives {~0, 1})
  * steepest-ascent direction field over (value, -index) lexicographic order
  * basin labels resolved by iterated directional prefix scans
    (tensor_tensor_scan along rows; transposed for columns)
  * outputs: v field + basin label field (peak pixel index per pixel)

Host:
  * finish label convergence (pointer jumping; device normally converges)
  * contract each basin to its peak; boundary-pair edges w=min(v_p,v_q)
  * Kruskal union-find over ~1k peaks -> persistence bars (exactly equal to
    the reference's pixel-level union-find diagram; validated)
  * closed-form rank matching loss, mean over batch.
"""

import numpy as np

H = W = 64
N = H * W
NROUNDS = 8
NEG = -1e30
FALLBACKS = 0  # images where the host had to re-resolve labels from scratch

_NC_CACHE = {}
TRACE = False          # test harness can flip this to profile
LAST_RESULTS = None    # BassKernelResults of the most recent device run


def _build_nc():
    import concourse.bass as bass
    import concourse.bacc as bacc
    import concourse.mybir as mybir
    from concourse.tile import TileContext

    f32 = mybir.dt.float32
    Alu = mybir.AluOpType
    Act = mybir.ActivationFunctionType

    from concourse import masks as masks_mod

    nc = bacc.Bacc(None)
    x = nc.dram_tensor("x", [2, H, W], f32, kind="ExternalInput")
    # packed output: cols 0:64 = v field, cols 64:128 = transposed labels
    out = nc.dram_tensor("out", [H, 2 * W], f32, kind="ExternalOutput")

    with TileContext(nc) as tc:
        with (
            tc.tile_pool(name="main", bufs=1) as pool,
            tc.tile_pool(name="psum", bufs=2, space="PSUM") as psum,
        ):
            T = lambda name: pool.tile([H, W], f32, tag=name, name=name)

            # gpsimd work first so its queue drains during the input DMA:
            # iota (label init), border fills, cascade constants, identity
            ii = pool.tile([H, W], mybir.dt.int32, tag="ii", name="ii")
            nc.gpsimd.iota(ii[:], pattern=[[1, W]], base=0, channel_multiplier=W)
            vN = T("vN")
            vS = T("vS")
            nc.gpsimd.memset(vN[:], NEG)
            nc.gpsimd.memset(vS[:], NEG)
            bD = T("bD")
            consts = {}
            for code in (0.0, 1.0, 2.0):
                c = T(f"k{int(code)}")
                nc.gpsimd.memset(c[:], code)
                consts[code] = c
            ident = T("ident")
            masks_mod.make_identity(nc, ident[:])

            pack = pool.tile([H, 2 * W], f32, tag="pack", name="pack")
            v = pack[:, 0:W]

            xw = pool.tile([H, 2 * W], f32, tag="xw", name="xw")
            nc.sync.dma_start(
                xw[:], bass.AP(x, 0, [[W, H], [H * W, 2], [1, W]])
            )
            # warm the sigmoid table on ACT while the input DMA is in flight
            warm = pool.tile([H, 1], f32, tag="warm", name="warm")
            nc.vector.memset(warm[:], 0.0)
            nc.scalar.activation(warm[:], warm[:], Act.Sigmoid)

            d = T("d")
            nc.vector.tensor_tensor(d[:], xw[:, W : 2 * W], xw[:, 0:W], Alu.subtract)
            nc.scalar.activation(v, d[:], Act.Sigmoid)

            # neighbor-shifted value fields, NEG at borders
            vW = T("vW")
            vE = T("vE")
            nc.sync.dma_start(vN[1:H, :], v[0 : H - 1, :])
            nc.sync.dma_start(vS[0 : H - 1, :], v[1:H, :])
            nc.vector.memset(vW[:, 0:1], NEG)
            nc.vector.tensor_copy(vW[:, 1:W], v[:, 0 : W - 1])
            nc.vector.memset(vE[:, W - 1 : W], NEG)
            nc.vector.tensor_copy(vE[:, 0 : W - 1], v[:, 1:W])

            # lexicographic argmax over (value, -index), processed in
            # DECREASING index order with >= so smaller indices win ties.
            # Phase A uses only DMA-free candidates {E, self, W} so it can
            # run while the N/S shift DMAs are in flight.
            bV = T("bV")
            t = pool.tile([H, W], mybir.dt.uint32, tag="t", name="t")
            nc.vector.tensor_copy(bV[:], vE[:])
            nc.gpsimd.memset(bD[:], 3.0)
            for cand, code in ((v, 0.0), (vW, 2.0)):
                nc.vector.tensor_tensor(t[:], cand[:], bV[:], Alu.is_ge)
                nc.vector.copy_predicated(bV[:], t[:], cand[:])
                nc.vector.copy_predicated(bD[:], t[:], consts[code][:])
            # Phase B: merge S (largest index, loses ties) and N (smallest,
            # wins ties) around the phase-A result.
            bV2 = T("bV2")
            bD2 = T("bD2")
            nc.vector.tensor_copy(bV2[:], vS[:])
            nc.gpsimd.memset(bD2[:], 4.0)
            nc.vector.tensor_tensor(t[:], bV[:], bV2[:], Alu.is_ge)
            nc.vector.copy_predicated(bV2[:], t[:], bV[:])
            nc.vector.copy_predicated(bD2[:], t[:], bD[:])
            nc.vector.tensor_tensor(t[:], vN[:], bV2[:], Alu.is_ge)
            nc.vector.copy_predicated(bD2[:], t[:], consts[1.0][:])
            bD = bD2

            # row-space masks + complements
            dirmask = {}
            for code, name in ((2.0, "mW"), (3.0, "mE")):
                m = T(name)
                nc.vector.tensor_scalar(m[:], bD[:], code, None, Alu.is_equal)
                nm = T("n" + name)
                nc.vector.tensor_scalar(nm[:], m[:], -1.0, 1.0, Alu.mult, Alu.add)
                dirmask[name] = m
                dirmask["n" + name] = nm
            # col-space masks from PE-transposed direction field
            bDT = psum.tile([H, W], f32, tag="bDT", name="bDT")
            nc.tensor.transpose(bDT[:], bD[:], ident[:])
            for code, name in ((1.0, "mNT"), (4.0, "mST")):
                m = T(name)
                nc.vector.tensor_scalar(m[:], bDT[:], code, None, Alu.is_equal)
                nm = T("n" + name)
                nc.vector.tensor_scalar(nm[:], m[:], -1.0, 1.0, Alu.mult, Alu.add)
                dirmask[name] = m
                dirmask["n" + name] = nm
            mW_, nmW_ = dirmask["mW"], dirmask["nmW"]
            mE_, nmE_ = dirmask["mE"], dirmask["nmE"]
            mNT_, nmNT_ = dirmask["mNT"], dirmask["nmNT"]
            mST_, nmST_ = dirmask["mST"], dirmask["nmST"]

            # label init = own pixel index
            Li0 = T("Li0")
            nc.vector.tensor_copy(Li0[:], ii[:])

            tmp = T("tmp")
            La = T("La")
            Lb = T("Lb")
            Lc = T("Lc")
            Ld = pack[:, W : 2 * W]
            cur = Li0[:]
            for r in range(NROUNDS):
                # W-chains: left->right scan along rows
                nc.vector.tensor_tensor(tmp[:], cur, nmW_[:], Alu.mult)
                nc.vector.tensor_tensor_scan(
                    La[:], mW_[:], tmp[:], 0.0, Alu.mult, Alu.add
                )
                # E-chains: right->left scan (reversed views)
                nc.vector.tensor_tensor(tmp[:], La[:], nmE_[:], Alu.mult)
                nc.vector.tensor_tensor_scan(
                    Lb[:, ::-1], mE_[:, ::-1], tmp[:, ::-1], 0.0, Alu.mult, Alu.add
                )
                # to column space on the PE
                psT = psum.tile([H, W], f32, tag="psT", name="psT")
                nc.tensor.transpose(psT[:], Lb[:], ident[:])
                # N-chains: left->right in transposed space
                nc.vector.tensor_tensor(tmp[:], psT[:], nmNT_[:], Alu.mult)
                nc.vector.tensor_tensor_scan(
                    Lc[:], mNT_[:], tmp[:], 0.0, Alu.mult, Alu.add
                )
                # S-chains: right->left in transposed space
                nc.vector.tensor_tensor(tmp[:], Lc[:], nmST_[:], Alu.mult)
                last = r == NROUNDS - 1
                sout = Ld if last else T("Ls")
                nc.vector.tensor_tensor_scan(
                    sout[:, ::-1], mST_[:, ::-1], tmp[:, ::-1],
                    0.0, Alu.mult, Alu.add,
                )
                if not last:
                    # back to row space for the next round
                    psR = psum.tile([H, W], f32, tag="psR", name="psR")
                    nc.tensor.transpose(psR[:], sout[:], ident[:])
                    cur = psR[:]

            nc.sync.dma_start(out[:], pack[:])

    return nc


def _run_device(xs):
    """xs: list of 8 arrays [2,H,W] f32. Returns list of (v, li) pairs."""
    from concourse.bass_utils import run_bass_kernel_spmd

    if "nc" not in _NC_CACHE:
        nc = _build_nc()
        if not nc.is_finalized():
            nc.finalize()
        _NC_CACHE["nc"] = nc
    nc = _NC_CACHE["nc"]
    res = run_bass_kernel_spmd(
        nc,
        [{"x": np.ascontiguousarray(x, dtype=np.float32)} for x in xs],
        core_ids=list(range(8)),
        trace=TRACE,
    )
    global LAST_RESULTS
    LAST_RESULTS = res
    # packed output: cols 0:64 = v, cols 64:128 = labels in transposed layout
    return [
        (r["out"][:, 0:W], np.ascontiguousarray(r["out"][:, W : 2 * W].T))
        for r in res.results
    ]


# ---------------------------------------------------------------------------
# host post-processing
# ---------------------------------------------------------------------------

def _ascent_ptr(v):
    """Pointer to steepest-ascent target under (value, -index) lex order.
    Must mirror the device compare cascade bit-exactly (pure f32 compares)."""
    neg = np.float32(NEG)
    vN = np.full((H, W), neg, np.float32); vN[1:, :] = v[:-1, :]
    vS = np.full((H, W), neg, np.float32); vS[:-1, :] = v[1:, :]
    vW = np.full((H, W), neg, np.float32); vW[:, 1:] = v[:, :-1]
    vE = np.full((H, W), neg, np.float32); vE[:, :-1] = v[:, 1:]
    bV = vN.copy()
    bD = np.full((H, W), 1, np.int32)
    for cand, code in ((vW, 2), (v, 0), (vE, 3), (vS, 4)):
        take = cand > bV
        bV = np.where(take, cand, bV)
        bD = np.where(take, code, bD)
    idx = np.arange(N).reshape(H, W)
    off = np.array([0, -W, -1, 1, W])
    return (idx + off[bD]).reshape(-1)


def _resolve_labels(li, ptr):
    """Finish pointer-jumping from the device's (normally converged) labels,
    then verify against the ascent forest; fall back to exact pointer
    resolution if the device field is inconsistent."""
    L = li
    for _ in range(14):
        L2 = L[L]
        if np.array_equal(L2, L):
            break
        L = L2
    # validity: constant along ascent edges, and ascent roots self-labeled
    ok = np.array_equal(L, L[ptr])
    if ok:
        roots = ptr == np.arange(N)
        ok = np.array_equal(L[roots], np.arange(N)[roots])
    if ok:
        return L
    global FALLBACKS
    FALLBACKS += 1
    L = ptr
    while True:
        L2 = L[L]
        if np.array_equal(L2, L):
            return L
        L = L2


def _diagram(v, L):
    """Positive-persistence bars via basin contraction + Kruskal."""
    vf = v.reshape(-1).astype(np.float64)
    Lg = L.reshape(H, W)
    vg = v.reshape(H, W).astype(np.float64)

    eu = np.concatenate([Lg[:, :-1].reshape(-1), Lg[:-1, :].reshape(-1)])
    ev = np.concatenate([Lg[:, 1:].reshape(-1), Lg[1:, :].reshape(-1)])
    ew = np.concatenate([
        np.minimum(vg[:, :-1], vg[:, 1:]).reshape(-1),
        np.minimum(vg[:-1, :], vg[1:, :]).reshape(-1),
    ])
    m = eu != ev
    eu, ev, ew = eu[m], ev[m], ew[m]
    # one edge per unordered basin pair: keep the max weight
    lo = np.minimum(eu, ev)
    hi = np.maximum(eu, ev)
    order = np.lexsort((-ew, hi, lo))
    lo, hi, ew = lo[order], hi[order], ew[order]
    first = np.ones(len(lo), dtype=bool)
    first[1:] = (lo[1:] != lo[:-1]) | (hi[1:] != hi[:-1])
    lo, hi, ew = lo[first], hi[first], ew[first]
    # Kruskal by decreasing weight
    order = np.argsort(-ew, kind="stable")
    lo, hi, ew = lo[order], hi[order], ew[order]

    peaks = np.unique(L)
    pid = np.full(N, -1, np.int64)
    pid[peaks] = np.arange(len(peaks))
    parent = np.arange(len(peaks))
    birth = vf[peaks]

    plist = parent
    bars_b = []
    bars_d = []

    def find(i):
        while plist[i] != i:
            plist[i] = plist[plist[i]]
            i = plist[i]
        return i

    merges = 0
    need = len(peaks) - 1
    for k in range(len(ew)):
        ri = find(pid[lo[k]])
        rj = find(pid[hi[k]])
        if ri == rj:
            continue
        if birth[ri] >= birth[rj]:
            elder, young = ri, rj
        else:
            elder, young = rj, ri
        if birth[young] > ew[k]:
            bars_b.append(birth[young])
            bars_d.append(ew[k])
        plist[young] = elder
        merges += 1
        if merges == need:
            break
    vmax = vf.max()
    vmin = vf.min()
    if vmax > vmin:
        bars_b.append(vmax)
        bars_d.append(vmin)
    return np.array(bars_b), np.array(bars_d)


def _match_loss(b1, d1, b2, d2):
    p1 = b1 - d1
    p2 = b2 - d2
    o1 = np.argsort(-p1, kind="stable")
    o2 = np.argsort(-p2, kind="stable")
    b1, d1 = b1[o1], d1[o1]
    b2, d2 = b2[o2], d2[o2]
    K1, K2 = len(b1), len(b2)
    Km = min(K1, K2)
    loss = 0.0
    if Km:
        loss += np.sum((b1[:Km] - b2[:Km]) ** 2 + (d1[:Km] - d2[:Km]) ** 2)
    if K1 > Km:
        loss += 0.5 * np.sum((b1[Km:] - d1[Km:]) ** 2)
    if K2 > Km:
        loss += 0.5 * np.sum((b2[Km:] - d2[Km:]) ** 2)
    return loss


def _postprocess(v, li):
    v = np.asarray(v, np.float32).reshape(H, W)
    li = np.asarray(li).reshape(-1).astype(np.int64)
    ptr = _ascent_ptr(v)
    L = _resolve_labels(li, ptr)
    return _diagram(v, L)


def kernel(input, target):
    input = np.asarray(input, np.float32)
    target = np.asarray(target, np.float32)
    B = input.shape[0]
    assert B == 4 and input.shape == (4, 2, H, W) and target.shape == (4, H, W)

    xs = []
    for s in range(B):
        xs.append(input[s])
    for s in range(B):
        t = np.zeros((2, H, W), np.float32)
        t[1] = target[s] * np.float32(80.0) - np.float32(40.0)
        xs.append(t)

    outs = _run_device(xs)

    losses = []
    for s in range(B):
        bp, dp = _postprocess(*outs[s])
        bt, dt = _postprocess(*outs[4 + s])
        losses.append(_match_loss(bp, dp, bt, dt))
    return np.float32(np.mean(losses))
